# revision 49
# baseline (speedup 1.0000x reference)
"""MultiHeadSelfAttention + ALiBi for Trainium2, SPMD over 8 NeuronCores.

Sharding: core c handles batch b = c // 4 and head group g = c % 4
(3 of the 12 heads, one per ALiBi band class so per-core work balances).
Each core computes y_partial[b] = ctx(heads_g) @ Wout[rows_g]; the host
sums the 4 partials per batch and adds bout.

Device pipeline per core:
  1. QKV projection in bf16 (weights/x pre-cast on host).  Q'/8+bq and
     K+bk written as float32r into per-head attention operand buffers;
     V (+ones column for softmax denominators) kept in bf16.
  2. S^T blocks [128k x 512q] as float32r matmuls with 3 augmented
     contraction rows carrying the attention-mask bias and, off the
     diagonal, the exact ALiBi term -slope*|q-k| (linear there, indices
     re-centered at 1024 to bound fp32r rounding).  Diagonal blocks get
     a fused DVE (rel * -slope + S) pass instead.  exp() on ScalarE over
     3-block groups -> bf16 P^T; P^T @ V_aug (bf16) accumulated in PSUM
     -> unnormalized ctx^T + denominator row.  Per-slot key-tile bands
     skip blocks where ALiBi decays attention below ~e^-18.
  3. Unnormalized ctx^T (bf16) + denominators staged to SBUF; one
     batched Ln + Exp(-x) pass produces 1/denom (single ACT table set);
     reciprocals broadcast across partitions via K=1 matmuls; ctx
     normalized in place; y = ctx^T.T @ Wout rows (bf16) -> fp32 out.
"""

import math
import os

import numpy as np


def _ensure_concourse():
    try:
        import concourse  # noqa: F401
    except ImportError:
        import sys

        for p in ("/opt/trn_rl_repo", "/root/.axon_site/_ro/trn_rl_repo"):
            if os.path.isdir(p) and p not in sys.path:
                sys.path.insert(0, p)


B, L, D, H, DH = 2, 2048, 768, 12, 64
KT = L // 128  # 16 k-tiles
QC = L // 512  # 4 q-chunks
NH = 3  # heads per core
N_CORES = 8
GROUP_SIZE = 3  # exp/S group size in k-tiles (3 PSUM banks)

NEG_MASK = -1.0e9
QCENTER = 1024.0  # index re-centering for fp32r aug rows

# Per head-slot key-tile bands per q-chunk (t_lo, t_hi_exclusive).  Heads
# are assigned to slots by band class; band d satisfies e^(-slope*d) <=
# e^-14 for every head in the slot, so skipped blocks contribute < 2e-3
# relative mass.  Slot 0: heads {7,6,5,4} (widest -> full); slot 1:
# heads {3,2,11,1} (d=224); slot 2: heads {10,9,0,8} (d=79).
BANDS = [
    [(0, 16), (0, 16), (0, 16), (0, 16)],  # slot 0: full
    [(0, 6), (2, 10), (6, 14), (10, 16)],  # slot 1: d=224
    [(0, 5), (3, 9), (7, 13), (11, 16)],  # slot 2: d=79
]

# One head per band class per group -> identical program on all cores.
HEAD_GROUPS = [[7, 3, 10], [6, 2, 9], [5, 11, 0], [4, 1, 8]]


def alibi_slopes(n_heads: int) -> np.ndarray:
    def slopes_pow2(n):
        start = 2 ** (-(2 ** -(math.log2(n) - 3)))
        return [start * start**i for i in range(n)]

    if math.log2(n_heads).is_integer():
        s = slopes_pow2(n_heads)
    else:
        cp = 2 ** int(math.floor(math.log2(n_heads)))
        s = slopes_pow2(cp) + slopes_pow2(2 * cp)[0::2][: n_heads - cp]
    return np.asarray(s, dtype=np.float32)


_PROGRAM_CACHE = {}


def _build_program():
    """Build the (shared, SPMD) Bass program once."""
    if "nc" in _PROGRAM_CACHE:
        return _PROGRAM_CACHE["nc"]

    _ensure_concourse()
    import concourse.mybir as mybir
    import concourse.tile as tile
    from concourse import bacc
    from concourse.bass import ts

    f32 = mybir.dt.float32
    f32r = mybir.dt.float32r
    bf16 = mybir.dt.bfloat16
    Exp = mybir.ActivationFunctionType.Exp
    Ln = mybir.ActivationFunctionType.Ln
    MULT = mybir.AluOpType.mult
    ADD = mybir.AluOpType.add

    nc = bacc.Bacc(None)

    # ---- DRAM I/O ----
    xT_d = nc.dram_tensor("xT", [D, L], bf16, kind="ExternalInput")
    wqk_d = nc.dram_tensor("wqk", [D, 2 * DH * NH], bf16, kind="ExternalInput")
    bqk_d = nc.dram_tensor("bqk", [128, NH], f32, kind="ExternalInput")
    wv_d = nc.dram_tensor("wv", [D, DH * NH], bf16, kind="ExternalInput")
    bv_d = nc.dram_tensor("bv", [1, DH * NH], bf16, kind="ExternalInput")
    woutp_d = nc.dram_tensor("woutp", [256, D], bf16, kind="ExternalInput")
    augqR_d = nc.dram_tensor("augqR", [4, L], f32r, kind="ExternalInput")
    augqL_d = nc.dram_tensor("augqL", [4, L], f32r, kind="ExternalInput")
    augk_d = nc.dram_tensor("augk", [NH, 4, L], f32r, kind="ExternalInput")
    # rel1[p, j, q''] = -slope_j * |q'' - p|: ALiBi for the 128-wide kinked
    # segment of diagonal blocks (the linear flanks use the aug rows).
    rel1_d = nc.dram_tensor("rel1", [128, NH, 128], f32, kind="ExternalInput")
    y_d = nc.dram_tensor("ypart", [L, D], f32, kind="ExternalOutput")

    with tile.TileContext(nc) as tc:
        with tc.tile_pool(name="persist", bufs=1) as pp:
            # ---- persistent SBUF ----
            wqk_sb = pp.tile([128, 6, 2 * DH * NH], bf16)
            bqk_sb = pp.tile([128, NH], f32)
            wv_sb = pp.tile([128, 6, DH * NH], bf16)
            bv_sb = pp.tile([1, DH * NH], bf16)
            woutp_sb = pp.tile([128, 2, D], bf16)
            rel1_sb = pp.tile([128, NH, 128], f32)
            ones_sb = pp.tile([65, 128], bf16)  # rows 0/32/64 used
            V_sb = pp.tile([128, KT, NH, DH + 1], bf16)
            ctxA = pp.tile([128, L], bf16)  # heads 0,1 of group (unnorm ctx^T)
            ctxB = pp.tile([64, L], bf16)  # head 2 of group
            # softmax denominators / reciprocals: head j lives on partition
            # 32*j so the K=1 broadcast matmul sees a legal base partition.
            den_sb = pp.tile([65, L], f32)
            lnd_sb = pp.tile([65, L], f32)
            rec_sb = pp.tile([65, L], bf16)
            # Per-head attention operand buffers (fp32r).
            # The ALiBi k-term is split hi+lo (hi bf16-exact) so the PE's
            # reduced-precision fp32r input rounding cannot distort it.
            # Kbuf rows: 0-63 K^T, 64 mask, 65 hi(s*(k-1024)), 66 lo, 67 s
            # QbufR rows: 0-63 Q'^T, 64 1, 65 1, 66 1, 67 -(q-1024)
            # QbufL rows: 0-63 Q'^T, 64 1, 65 -1, 66 -1, 67 +(q-1024)
            Kbuf = [
                pp.tile([68, L], f32r, tag=f"kb{j}", name=f"kb{j}") for j in range(NH)
            ]
            QbufR = [
                pp.tile([68, L], f32r, tag=f"qr{j}", name=f"qr{j}") for j in range(NH)
            ]
            QbufL = [
                pp.tile([68, L], f32r, tag=f"ql{j}", name=f"ql{j}") for j in range(NH)
            ]

            # Stage-1-critical DMAs first (x/wqk feed the first matmuls,
            # interleaved so the kt=0 pair lands earliest); later-needed
            # tensors go on the gpsimd DMA queue so they don't delay the
            # compute-critical loads.
            wqk_r = wqk_d.rearrange("(o p) m -> p o m", p=128)
            xT_r = xT_d.rearrange("(o p) f -> p o f", p=128)
            xT_sb = pp.tile([128, 6, L], bf16, name="xT_sb")
            for kt in range(6):
                nc.sync.dma_start(xT_sb[:, kt, :], xT_r[:, kt, :])
                nc.sync.dma_start(wqk_sb[:, kt, :], wqk_r[:, kt, :])
            nc.sync.dma_start(bqk_sb[:], bqk_d[:])
            nc.sync.dma_start(wv_sb[:], wv_d.rearrange("(o p) m -> p o m", p=128))
            nc.sync.dma_start(bv_sb[:], bv_d[:])
            nc.vector.memset(ones_sb[:], 1.0)
            nc.gpsimd.memset(V_sb[:, :, :, DH : DH + 1], 1.0)
            # unused den rows must stay finite through the batched Ln/Exp
            nc.gpsimd.memset(den_sb[:], 1.0)
            for j in range(NH):
                nc.gpsimd.dma_start(QbufR[j][64:68, :], augqR_d[:])
                nc.gpsimd.dma_start(QbufL[j][64:68, :], augqL_d[:])
                nc.gpsimd.dma_start(Kbuf[j][64:68, :], augk_d[j])
            nc.gpsimd.dma_start(rel1_sb[:], rel1_d[:])
            nc.gpsimd.dma_start(
                woutp_sb[:], woutp_d.rearrange("(o p) n -> p o n", p=128)
            )

            # Pre-touch DMA/memset-filled tiles with the engine that later
            # writes other rows of the same tile, so those writes carry a
            # single sync wait (walrus TS encoding rejects multi-wait).
            junk = pp.tile([1, 16], f32, name="junk")
            nc.vector.tensor_copy(junk[0:1, 0:1], bqk_sb[0:1, 0:1])
            nc.vector.tensor_copy(junk[0:1, 1:2], rel1_sb[0:1, 0, 0:1])
            junkr = pp.tile([1, 16], f32r, name="junkr")
            for j in range(NH):
                nc.vector.tensor_copy(junkr[0:1, 2 + j : 3 + j], QbufR[j][64:65, 0:1])
                nc.vector.tensor_copy(junkr[0:1, 5 + j : 6 + j], QbufL[j][64:65, 0:1])
                nc.vector.tensor_copy(junkr[0:1, 8 + j : 9 + j], Kbuf[j][64:65, 0:1])
            junkb = pp.tile([1, 4], bf16, name="junkb")
            nc.vector.tensor_copy(junkb[0:1, 0:1], V_sb[0:1, 0, 0, DH : DH + 1])

            # ---- stage 1: QKV projection (bf16 matmuls) ----
            with tc.tile_pool(name="ps1", bufs=4, space="PSUM") as ps1:
                # Q^T/K^T per head: PSUM [128, 512] = [Q^T_h/8; K^T_h] chunk
                # (the 1/8 scale is folded into wqk/bqk host-side)
                for j in range(NH):
                    pcs = [
                        ps1.tile([128, 512], f32, tag="ps1", name=f"ps1c{c}")
                        for c in range(QC)
                    ]
                    for kt in range(6):
                        for c in range(QC):
                            nc.tensor.matmul(
                                pcs[c],
                                wqk_sb[:, kt, ts(j, 128)],
                                xT_sb[:, kt, ts(c, 512)],
                                start=(kt == 0),
                                stop=(kt == 5),
                            )
                    for c in range(QC):
                        ps = pcs[c]
                        cs = ts(c, 512)
                        nc.vector.tensor_scalar(
                            QbufR[j][0:64, cs],
                            ps[0:64, :],
                            bqk_sb[0:64, j : j + 1],
                            None,
                            ADD,
                        )
                        # Q rows are identical in both side-buffers; replicate
                        # via DMA to keep the DVE free.
                        nc.sync.dma_start(QbufL[j][0:64, cs], QbufR[j][0:64, cs])
                        nc.vector.tensor_scalar(
                            Kbuf[j][0:64, cs],
                            ps[64:128, :],
                            bqk_sb[64:128, j : j + 1],
                            None,
                            ADD,
                        )

                # V natural layout [l, d] + bias via K=1 matmul
                for lt in range(KT):
                    psv = ps1.tile([128, 512], f32, tag="ps1", name="psv")[:, : DH * NH]
                    for kt in range(6):
                        nc.tensor.matmul(
                            psv,
                            xT_sb[:, kt, ts(lt, 128)],
                            wv_sb[:, kt, :],
                            start=(kt == 0),
                            stop=False,
                        )
                    nc.tensor.matmul(
                        psv,
                        ones_sb[0:1, 0:128],
                        bv_sb[0:1, :],
                        start=False,
                        stop=True,
                    )
                    nc.vector.tensor_copy(
                        V_sb[:, lt, :, 0:DH],
                        psv.rearrange("p (h x) -> p h x", x=DH),
                    )

            # ---- stage 2: attention ----
            with (
                tc.tile_pool(name="psS", bufs=2, space="PSUM") as psS,
                tc.tile_pool(name="psO", bufs=2, space="PSUM") as psO,
                tc.tile_pool(name="ptp", bufs=3) as ptp,
            ):
                for j in range(NH):
                    for c in range(QC):
                        cs = ts(c, 512)
                        t_lo, t_hi = BANDS[j][c]
                        out_t = psO.tile([128, 512], f32, tag="outaug")
                        for t0 in range(t_lo, t_hi, GROUP_SIZE):
                            tn = min(GROUP_SIZE, t_hi - t0)
                            st = psS.tile([128, GROUP_SIZE * 512], f32, tag="st")
                            for i in range(tn):
                                t = t0 + i
                                js = ts(i, 512)
                                m = t - 4 * c
                                if 0 <= m < 4:
                                    # Diagonal block: the abs kink only hits
                                    # q' in [128m, 128m+128); the flanks are
                                    # linear and use the aug-row paths.
                                    lw = 128 * m  # flank where q < k
                                    for a, b, buf, nr in (
                                        (0, lw, QbufL[j], 68),
                                        (lw, lw + 128, QbufR[j], 65),
                                        (lw + 128, 512, QbufR[j], 68),
                                    ):
                                        if a == b:
                                            continue
                                        nc.tensor.matmul(
                                            st[:, 512 * i + a : 512 * i + b],
                                            Kbuf[j][0:nr, ts(t, 128)],
                                            buf[0:nr, 512 * c + a : 512 * c + b],
                                            start=True,
                                            stop=True,
                                        )
                                    ms = slice(512 * i + lw, 512 * i + lw + 128)
                                    nc.vector.scalar_tensor_tensor(
                                        st[:, ms],
                                        rel1_sb[:, j, :],
                                        1.0,
                                        st[:, ms],
                                        MULT,
                                        ADD,
                                    )
                                elif c > t // 4:  # k < q: aug gives -s*(q-k)
                                    nc.tensor.matmul(
                                        st[:, js],
                                        Kbuf[j][0:68, ts(t, 128)],
                                        QbufR[j][0:68, cs],
                                        start=True,
                                        stop=True,
                                    )
                                else:  # k > q: aug gives -s*(k-q)
                                    nc.tensor.matmul(
                                        st[:, js],
                                        Kbuf[j][0:68, ts(t, 128)],
                                        QbufL[j][0:68, cs],
                                        start=True,
                                        stop=True,
                                    )
                            pt = ptp.tile([128, GROUP_SIZE * 512], bf16, tag="pt")
                            nc.scalar.activation(
                                pt[:, : tn * 512], st[:, : tn * 512], Exp
                            )
                            for i in range(tn):
                                t = t0 + i
                                nc.tensor.matmul(
                                    out_t[0 : DH + 1, :],
                                    V_sb[:, t, j, :],
                                    pt[:, ts(i, 512)],
                                    start=(t == t_lo),
                                    stop=(t == t_hi - 1),
                                    skip_group_check=True,
                                )
                        # stage unnormalized ctx + denominator to SBUF
                        if j < 2:
                            ctx_slice = ctxA[j * 64 : (j + 1) * 64, cs]
                        else:
                            ctx_slice = ctxB[0:64, cs]
                        nc.vector.tensor_copy(ctx_slice, out_t[0:DH, :])
                        nc.vector.tensor_copy(
                            den_sb[32 * j : 32 * j + 1, cs], out_t[DH : DH + 1, :]
                        )

            # ---- stage 3: normalization + output projection ----
            with (
                tc.tile_pool(name="ps3", bufs=2, space="PSUM") as ps3,
                tc.tile_pool(name="psR", bufs=2, space="PSUM") as psR,
                tc.tile_pool(name="ysb", bufs=3) as yp,
            ):
                # Keep the PE busy (HAM warm) through the reciprocal chain:
                # these matmuls depend on den_sb, so they schedule exactly
                # into the Ln/Exp window the PE would otherwise idle in.
                # (read the last-written den row/chunk so these only become
                # ready once stage 2 fully drains; bf16 bitcast keeps each
                # at ~213ns so the total matches the Ln/Exp latency)
                dum = psR.tile([128, 512], f32, tag="dum")
                dk = den_sb[64:65, 1536:2048].bitcast(bf16)
                for _ in range(16):
                    nc.tensor.matmul(
                        dum,
                        dk[0:1, 0:128],
                        dk[0:1, 0:512],
                        start=True,
                        stop=True,
                        skip_group_check=True,
                    )
                # 1/denom = exp(-ln(denom)), one batched pass (Ln and Exp
                # share the natural_log_exp ACT table set).
                nc.scalar.activation(lnd_sb[:], den_sb[:], Ln)
                nc.scalar.activation(rec_sb[:], lnd_sb[:], Exp, scale=-1.0)

                # broadcast 1/denom across 64 partitions via K=1 matmuls,
                # then normalize ctx in place against the PSUM broadcast.
                for j in range(NH):
                    for c in range(QC):
                        cs = ts(c, 512)
                        recb_ps = psR.tile([64, 512], f32, tag="recb")
                        nc.tensor.matmul(
                            recb_ps,
                            ones_sb[32 * j : 32 * j + 1, 0:64],
                            rec_sb[32 * j : 32 * j + 1, cs],
                            start=True,
                            stop=True,
                        )
                        if j < 2:
                            ctx_slice = ctxA[j * 64 : (j + 1) * 64, cs]
                        else:
                            ctx_slice = ctxB[0:64, cs]
                        nc.vector.tensor_mul(ctx_slice, ctx_slice, recb_ps)

                for lt in range(KT):
                    y = yp.tile([128, D], f32, tag="y")
                    for n0, nw in ((0, 512), (512, 256)):
                        ps = ps3.tile([128, 512], f32, tag="ps3", name="ps3t")[:, :nw]
                        nc.tensor.matmul(
                            ps,
                            ctxA[:, ts(lt, 128)],
                            woutp_sb[:, 0, n0 : n0 + nw],
                            start=True,
                            stop=False,
                        )
                        nc.tensor.matmul(
                            ps,
                            ctxB[0:64, ts(lt, 128)],
                            woutp_sb[0:64, 1, n0 : n0 + nw],
                            start=False,
                            stop=True,
                        )
                        if n0 == 0:
                            nc.scalar.copy(y[:, n0 : n0 + nw], ps)
                        else:
                            nc.vector.tensor_copy(y[:, n0 : n0 + nw], ps)
                    nc.sync.dma_start(y_d[ts(lt, 128), :], y)

    if not nc.is_finalized():
        nc.finalize()
    _PROGRAM_CACHE["nc"] = nc
    return nc


def _host_inputs(x, attn_mask, Wqkv, bqkv, Wout, bout):
    """Build the 8 per-core input dicts."""
    import ml_dtypes

    bf16 = ml_dtypes.bfloat16
    slopes = alibi_slopes(H)
    x = np.asarray(x, dtype=np.float32)
    attn_mask = np.asarray(attn_mask)
    Wqkv = np.asarray(Wqkv, dtype=np.float32)
    bqkv = np.asarray(bqkv, dtype=np.float32)
    Wout = np.asarray(Wout, dtype=np.float32)
    bout = np.asarray(bout, dtype=np.float32)

    q_idx = np.arange(L, dtype=np.float32)
    ones_row = np.ones(L, dtype=np.float32)
    qrel = q_idx - QCENTER
    augqR = np.ascontiguousarray(np.stack([ones_row, ones_row, ones_row, -qrel]))
    augqL = np.ascontiguousarray(np.stack([ones_row, -ones_row, -ones_row, qrel]))

    # rel1[p, q''] = |q'' - p|  (kinked-segment relative distance)
    p = np.arange(128, dtype=np.float32)[:, None]
    qq = np.arange(128, dtype=np.float32)[None, :]
    rel1_base = np.abs(qq - p).astype(np.float32)

    in_maps = []
    for core in range(N_CORES):
        b = core // 4
        g = core % 4
        heads = HEAD_GROUPS[g]

        wqk = np.empty((D, 2 * DH * NH), np.float32)
        bqk = np.empty((128, NH), np.float32)
        wv = np.empty((D, DH * NH), np.float32)
        bv = np.empty((1, DH * NH), np.float32)
        woutp = np.zeros((256, D), np.float32)
        augk = np.empty((NH, 4, L), np.float32)
        mask_row = np.where(attn_mask[b] == 0, NEG_MASK, 0.0).astype(np.float32)
        rel1 = np.empty((128, NH, 128), np.float32)
        for jj, h in enumerate(heads):
            rel1[:, jj] = -float(slopes[h]) * rel1_base
            # Q half pre-scaled by 1/8 = 1/sqrt(DH)
            wqk[:, jj * 128 : jj * 128 + 64] = Wqkv[:, h * DH : (h + 1) * DH] * 0.125
            wqk[:, jj * 128 + 64 : (jj + 1) * 128] = Wqkv[
                :, D + h * DH : D + (h + 1) * DH
            ]
            bqk[0:64, jj] = bqkv[h * DH : (h + 1) * DH] * 0.125
            bqk[64:128, jj] = bqkv[D + h * DH : D + (h + 1) * DH]
            wv[:, jj * DH : (jj + 1) * DH] = Wqkv[
                :, 2 * D + h * DH : 2 * D + (h + 1) * DH
            ]
            bv[0, jj * DH : (jj + 1) * DH] = bqkv[2 * D + h * DH : 2 * D + (h + 1) * DH]
            woutp[jj * DH : (jj + 1) * DH, :] = Wout[h * DH : (h + 1) * DH, :]
            s = float(slopes[h])
            kterm = s * qrel  # s * (k_idx - 1024)
            hi = kterm.astype(bf16).astype(np.float32)
            augk[jj, 0, :] = mask_row
            augk[jj, 1, :] = hi
            augk[jj, 2, :] = kterm - hi
            augk[jj, 3, :] = s
        in_maps.append(
            {
                "xT": np.ascontiguousarray(x[b].T).astype(bf16),
                "wqk": wqk.astype(bf16),
                "bqk": bqk,
                "wv": wv.astype(bf16),
                "bv": bv.astype(bf16),
                "woutp": woutp.astype(bf16),
                "augqR": augqR,
                "augqL": augqL,
                "augk": augk,
                "rel1": rel1,
            }
        )
    return in_maps


def kernel(x, attn_mask, Wqkv, bqkv, Wout, bout):
    _ensure_concourse()
    from concourse.bass_utils import run_bass_kernel_spmd

    nc = _build_program()
    in_maps = _host_inputs(x, attn_mask, Wqkv, bqkv, Wout, bout)

    res = run_bass_kernel_spmd(
        nc,
        in_maps,
        list(range(N_CORES)),
        trace=bool(os.environ.get("BASS_TRACE")),
    )
    outs = [r["ypart"] for r in res.results]
    out = np.zeros((B, L, D), np.float32)
    for core in range(N_CORES):
        out[core // 4] += outs[core]
    out += np.asarray(bout, np.float32)[None, None, :]
    if res.exec_time_ns is not None:
        kernel.last_exec_time_ns = res.exec_time_ns
    kernel.last_result = res
    return out


# revision 50
# speedup vs baseline: 1.0062x; 1.0062x over previous
"""MultiHeadSelfAttention + ALiBi for Trainium2, SPMD over 8 NeuronCores.

Sharding: core c handles batch b = c // 4 and head group g = c % 4
(3 of the 12 heads, one per ALiBi band class so per-core work balances).
Each core computes y_partial[b] = ctx(heads_g) @ Wout[rows_g]; the host
sums the 4 partials per batch and adds bout.

Device pipeline per core:
  1. QKV projection in bf16 (weights/x pre-cast on host).  Q'/8+bq and
     K+bk written as float32r into per-head attention operand buffers;
     V (+ones column for softmax denominators) kept in bf16.
  2. S^T blocks [128k x 512q] as float32r matmuls with 3 augmented
     contraction rows carrying the attention-mask bias and, off the
     diagonal, the exact ALiBi term -slope*|q-k| (linear there, indices
     re-centered at 1024 to bound fp32r rounding).  Diagonal blocks get
     a fused DVE (rel * -slope + S) pass instead.  exp() on ScalarE over
     3-block groups -> bf16 P^T; P^T @ V_aug (bf16) accumulated in PSUM
     -> unnormalized ctx^T + denominator row.  Per-slot key-tile bands
     skip blocks where ALiBi decays attention below ~e^-18.
  3. Unnormalized ctx^T (bf16) + denominators staged to SBUF; one
     batched Ln + Exp(-x) pass produces 1/denom (single ACT table set);
     reciprocals broadcast across partitions via K=1 matmuls; ctx
     normalized in place; y = ctx^T.T @ Wout rows (bf16) -> fp32 out.
"""

import math
import os

import numpy as np


def _ensure_concourse():
    try:
        import concourse  # noqa: F401
    except ImportError:
        import sys

        for p in ("/opt/trn_rl_repo", "/root/.axon_site/_ro/trn_rl_repo"):
            if os.path.isdir(p) and p not in sys.path:
                sys.path.insert(0, p)


B, L, D, H, DH = 2, 2048, 768, 12, 64
KT = L // 128  # 16 k-tiles
QC = L // 512  # 4 q-chunks
NH = 3  # heads per core
N_CORES = 8
GROUP_SIZE = 3  # exp/S group size in k-tiles (3 PSUM banks)

NEG_MASK = -1.0e9
QCENTER = 1024.0  # index re-centering for fp32r aug rows

# Per head-slot key-tile bands per q-chunk (t_lo, t_hi_exclusive).  Heads
# are assigned to slots by band class; band d satisfies e^(-slope*d) <=
# e^-14 for every head in the slot, so skipped blocks contribute < 2e-3
# relative mass.  Slot 0: heads {7,6,5,4} (widest -> full); slot 1:
# heads {3,2,11,1} (d=224); slot 2: heads {10,9,0,8} (d=79).
BANDS = [
    [(0, 16), (0, 16), (0, 16), (0, 16)],  # slot 0: full
    [(0, 6), (2, 10), (6, 14), (10, 16)],  # slot 1: d=224
    [(0, 5), (3, 9), (7, 13), (11, 16)],  # slot 2: d=79
]

# One head per band class per group -> identical program on all cores.
HEAD_GROUPS = [[7, 3, 10], [6, 2, 9], [5, 11, 0], [4, 1, 8]]


def alibi_slopes(n_heads: int) -> np.ndarray:
    def slopes_pow2(n):
        start = 2 ** (-(2 ** -(math.log2(n) - 3)))
        return [start * start**i for i in range(n)]

    if math.log2(n_heads).is_integer():
        s = slopes_pow2(n_heads)
    else:
        cp = 2 ** int(math.floor(math.log2(n_heads)))
        s = slopes_pow2(cp) + slopes_pow2(2 * cp)[0::2][: n_heads - cp]
    return np.asarray(s, dtype=np.float32)


_PROGRAM_CACHE = {}


def _build_program():
    """Build the (shared, SPMD) Bass program once."""
    if "nc" in _PROGRAM_CACHE:
        return _PROGRAM_CACHE["nc"]

    _ensure_concourse()
    import concourse.mybir as mybir
    import concourse.tile as tile
    from concourse import bacc
    from concourse.bass import ts

    f32 = mybir.dt.float32
    f32r = mybir.dt.float32r
    bf16 = mybir.dt.bfloat16
    Exp = mybir.ActivationFunctionType.Exp
    Ln = mybir.ActivationFunctionType.Ln
    MULT = mybir.AluOpType.mult
    ADD = mybir.AluOpType.add

    nc = bacc.Bacc(None)

    # ---- DRAM I/O ----
    xT_d = nc.dram_tensor("xT", [D, L], bf16, kind="ExternalInput")
    wqk_d = nc.dram_tensor("wqk", [D, 2 * DH * NH], bf16, kind="ExternalInput")
    bqk_d = nc.dram_tensor("bqk", [128, NH], f32, kind="ExternalInput")
    wv_d = nc.dram_tensor("wv", [D, DH * NH], bf16, kind="ExternalInput")
    bv_d = nc.dram_tensor("bv", [1, DH * NH], bf16, kind="ExternalInput")
    woutp_d = nc.dram_tensor("woutp", [256, D], bf16, kind="ExternalInput")
    augqR_d = nc.dram_tensor("augqR", [4, L], f32r, kind="ExternalInput")
    augqL_d = nc.dram_tensor("augqL", [4, L], f32r, kind="ExternalInput")
    augk_d = nc.dram_tensor("augk", [NH, 4, L], f32r, kind="ExternalInput")
    # rel1[p, j, q''] = -slope_j * |q'' - p|: ALiBi for the 128-wide kinked
    # segment of diagonal blocks (the linear flanks use the aug rows).
    rel1_d = nc.dram_tensor("rel1", [128, NH, 128], f32, kind="ExternalInput")
    y_d = nc.dram_tensor("ypart", [L, D], f32, kind="ExternalOutput")

    with tile.TileContext(nc) as tc:
        with tc.tile_pool(name="persist", bufs=1) as pp:
            # ---- persistent SBUF ----
            wqk_sb = pp.tile([128, 6, 2 * DH * NH], bf16)
            bqk_sb = pp.tile([128, NH], f32)
            wv_sb = pp.tile([128, 6, DH * NH], bf16)
            bv_sb = pp.tile([1, DH * NH], bf16)
            woutp_sb = pp.tile([128, 2, D], bf16)
            rel1_sb = pp.tile([128, NH, 128], f32)
            ones_sb = pp.tile([65, 128], bf16)  # rows 0/32/64 used
            V_sb = pp.tile([128, KT, NH, DH + 1], bf16)
            ctxA = pp.tile([128, L], bf16)  # heads 0,1 of group (unnorm ctx^T)
            ctxB = pp.tile([64, L], bf16)  # head 2 of group
            # softmax denominators / reciprocals: head j lives on partition
            # 32*j so the K=1 broadcast matmul sees a legal base partition.
            den_sb = pp.tile([65, L], f32)
            lnd_sb = pp.tile([65, L], f32)
            rec_sb = pp.tile([65, L], bf16)
            # Per-head attention operand buffers (fp32r).
            # The ALiBi k-term is split hi+lo (hi bf16-exact) so the PE's
            # reduced-precision fp32r input rounding cannot distort it.
            # Kbuf rows: 0-63 K^T, 64 mask, 65 hi(s*(k-1024)), 66 lo, 67 s
            # QbufR rows: 0-63 Q'^T, 64 1, 65 1, 66 1, 67 -(q-1024)
            # QbufL rows: 0-63 Q'^T, 64 1, 65 -1, 66 -1, 67 +(q-1024)
            Kbuf = [
                pp.tile([68, L], f32r, tag=f"kb{j}", name=f"kb{j}") for j in range(NH)
            ]
            QbufR = [
                pp.tile([68, L], f32r, tag=f"qr{j}", name=f"qr{j}") for j in range(NH)
            ]
            QbufL = [
                pp.tile([68, L], f32r, tag=f"ql{j}", name=f"ql{j}") for j in range(NH)
            ]

            # Stage-1-critical DMAs first (x/wqk feed the first matmuls,
            # interleaved so the kt=0 pair lands earliest); later-needed
            # tensors go on the gpsimd DMA queue so they don't delay the
            # compute-critical loads.
            wqk_r = wqk_d.rearrange("(o p) m -> p o m", p=128)
            xT_r = xT_d.rearrange("(o p) f -> p o f", p=128)
            xT_sb = pp.tile([128, 6, L], bf16, name="xT_sb")
            for kt in range(6):
                nc.sync.dma_start(xT_sb[:, kt, :], xT_r[:, kt, :])
                nc.sync.dma_start(wqk_sb[:, kt, :], wqk_r[:, kt, :])
            nc.sync.dma_start(bqk_sb[:], bqk_d[:])
            nc.sync.dma_start(wv_sb[:], wv_d.rearrange("(o p) m -> p o m", p=128))
            nc.sync.dma_start(bv_sb[:], bv_d[:])
            nc.vector.memset(ones_sb[:], 1.0)
            nc.gpsimd.memset(V_sb[:, :, :, DH : DH + 1], 1.0)
            # unused den rows must stay finite through the batched Ln/Exp
            nc.gpsimd.memset(den_sb[:], 1.0)
            for j in range(NH):
                nc.gpsimd.dma_start(QbufR[j][64:68, :], augqR_d[:])
                nc.gpsimd.dma_start(QbufL[j][64:68, :], augqL_d[:])
                nc.gpsimd.dma_start(Kbuf[j][64:68, :], augk_d[j])
            nc.gpsimd.dma_start(rel1_sb[:], rel1_d[:])
            nc.gpsimd.dma_start(
                woutp_sb[:], woutp_d.rearrange("(o p) n -> p o n", p=128)
            )

            # Pre-touch DMA/memset-filled tiles with the engine that later
            # writes other rows of the same tile, so those writes carry a
            # single sync wait (walrus TS encoding rejects multi-wait).
            junk = pp.tile([1, 16], f32, name="junk")
            nc.vector.tensor_copy(junk[0:1, 0:1], bqk_sb[0:1, 0:1])
            nc.vector.tensor_copy(junk[0:1, 1:2], rel1_sb[0:1, 0, 0:1])
            junkr = pp.tile([1, 16], f32r, name="junkr")
            for j in range(NH):
                nc.vector.tensor_copy(junkr[0:1, 2 + j : 3 + j], QbufR[j][64:65, 0:1])
                nc.vector.tensor_copy(junkr[0:1, 5 + j : 6 + j], QbufL[j][64:65, 0:1])
                nc.vector.tensor_copy(junkr[0:1, 8 + j : 9 + j], Kbuf[j][64:65, 0:1])
            junkb = pp.tile([1, 4], bf16, name="junkb")
            nc.vector.tensor_copy(junkb[0:1, 0:1], V_sb[0:1, 0, 0, DH : DH + 1])

            # ---- stage 1: QKV projection (bf16 matmuls) ----
            with tc.tile_pool(name="ps1", bufs=4, space="PSUM") as ps1:
                # Q^T/K^T per head: PSUM [128, 512] = [Q^T_h/8; K^T_h] chunk
                # (the 1/8 scale is folded into wqk/bqk host-side)
                for j in range(NH):
                    pcs = [
                        ps1.tile([128, 512], f32, tag="ps1", name=f"ps1c{c}")
                        for c in range(QC)
                    ]
                    for kt in range(6):
                        for c in range(QC):
                            nc.tensor.matmul(
                                pcs[c],
                                wqk_sb[:, kt, ts(j, 128)],
                                xT_sb[:, kt, ts(c, 512)],
                                start=(kt == 0),
                                stop=(kt == 5),
                            )
                    for c in range(QC):
                        ps = pcs[c]
                        cs = ts(c, 512)
                        nc.vector.tensor_scalar(
                            QbufR[j][0:64, cs],
                            ps[0:64, :],
                            bqk_sb[0:64, j : j + 1],
                            None,
                            ADD,
                        )
                        # Q rows are identical in both side-buffers; replicate
                        # via DMA to keep the DVE free.
                        nc.sync.dma_start(QbufL[j][0:64, cs], QbufR[j][0:64, cs])
                        nc.vector.tensor_scalar(
                            Kbuf[j][0:64, cs],
                            ps[64:128, :],
                            bqk_sb[64:128, j : j + 1],
                            None,
                            ADD,
                        )

                # V natural layout [l, d] + bias via K=1 matmul
                for lt in range(KT):
                    psv = ps1.tile([128, 512], f32, tag="ps1", name="psv")[:, : DH * NH]
                    for kt in range(6):
                        nc.tensor.matmul(
                            psv,
                            xT_sb[:, kt, ts(lt, 128)],
                            wv_sb[:, kt, :],
                            start=(kt == 0),
                            stop=False,
                        )
                    nc.tensor.matmul(
                        psv,
                        ones_sb[0:1, 0:128],
                        bv_sb[0:1, :],
                        start=False,
                        stop=True,
                    )
                    nc.vector.tensor_copy(
                        V_sb[:, lt, :, 0:DH],
                        psv.rearrange("p (h x) -> p h x", x=DH),
                    )

            # ---- stage 2: attention ----
            with (
                tc.tile_pool(name="psS", bufs=2, space="PSUM") as psS,
                tc.tile_pool(name="psO", bufs=2, space="PSUM") as psO,
                tc.tile_pool(name="ptp", bufs=4) as ptp,
            ):
                for j in range(NH):
                    for c in range(QC):
                        cs = ts(c, 512)
                        t_lo, t_hi = BANDS[j][c]
                        out_t = psO.tile([128, 512], f32, tag="outaug")
                        for t0 in range(t_lo, t_hi, GROUP_SIZE):
                            tn = min(GROUP_SIZE, t_hi - t0)
                            st = psS.tile([128, GROUP_SIZE * 512], f32, tag="st")
                            for i in range(tn):
                                t = t0 + i
                                js = ts(i, 512)
                                m = t - 4 * c
                                if 0 <= m < 4:
                                    # Diagonal block: the abs kink only hits
                                    # q' in [128m, 128m+128); the flanks are
                                    # linear and use the aug-row paths.
                                    lw = 128 * m  # flank where q < k
                                    for a, b, buf, nr in (
                                        (0, lw, QbufL[j], 68),
                                        (lw, lw + 128, QbufR[j], 65),
                                        (lw + 128, 512, QbufR[j], 68),
                                    ):
                                        if a == b:
                                            continue
                                        nc.tensor.matmul(
                                            st[:, 512 * i + a : 512 * i + b],
                                            Kbuf[j][0:nr, ts(t, 128)],
                                            buf[0:nr, 512 * c + a : 512 * c + b],
                                            start=True,
                                            stop=True,
                                        )
                                    ms = slice(512 * i + lw, 512 * i + lw + 128)
                                    nc.vector.scalar_tensor_tensor(
                                        st[:, ms],
                                        rel1_sb[:, j, :],
                                        1.0,
                                        st[:, ms],
                                        MULT,
                                        ADD,
                                    )
                                elif c > t // 4:  # k < q: aug gives -s*(q-k)
                                    nc.tensor.matmul(
                                        st[:, js],
                                        Kbuf[j][0:68, ts(t, 128)],
                                        QbufR[j][0:68, cs],
                                        start=True,
                                        stop=True,
                                    )
                                else:  # k > q: aug gives -s*(k-q)
                                    nc.tensor.matmul(
                                        st[:, js],
                                        Kbuf[j][0:68, ts(t, 128)],
                                        QbufL[j][0:68, cs],
                                        start=True,
                                        stop=True,
                                    )
                            pt = ptp.tile([128, GROUP_SIZE * 512], bf16, tag="pt")
                            nc.scalar.activation(
                                pt[:, : tn * 512], st[:, : tn * 512], Exp
                            )
                            for i in range(tn):
                                t = t0 + i
                                nc.tensor.matmul(
                                    out_t[0 : DH + 1, :],
                                    V_sb[:, t, j, :],
                                    pt[:, ts(i, 512)],
                                    start=(t == t_lo),
                                    stop=(t == t_hi - 1),
                                    skip_group_check=True,
                                )
                        # stage unnormalized ctx + denominator to SBUF
                        if j < 2:
                            ctx_slice = ctxA[j * 64 : (j + 1) * 64, cs]
                        else:
                            ctx_slice = ctxB[0:64, cs]
                        nc.vector.tensor_copy(ctx_slice, out_t[0:DH, :])
                        nc.vector.tensor_copy(
                            den_sb[32 * j : 32 * j + 1, cs], out_t[DH : DH + 1, :]
                        )

            # ---- stage 3: normalization + output projection ----
            with (
                tc.tile_pool(name="ps3", bufs=2, space="PSUM") as ps3,
                tc.tile_pool(name="psR", bufs=2, space="PSUM") as psR,
                tc.tile_pool(name="ysb", bufs=3) as yp,
            ):
                # Keep the PE busy (HAM warm) through the reciprocal chain:
                # these matmuls depend on den_sb, so they schedule exactly
                # into the Ln/Exp window the PE would otherwise idle in.
                # (read the last-written den row/chunk so these only become
                # ready once stage 2 fully drains; bf16 bitcast keeps each
                # at ~213ns so the total matches the Ln/Exp latency)
                dum = psR.tile([128, 512], f32, tag="dum")
                dk = den_sb[64:65, 1536:2048].bitcast(bf16)
                for _ in range(16):
                    nc.tensor.matmul(
                        dum,
                        dk[0:1, 0:128],
                        dk[0:1, 0:512],
                        start=True,
                        stop=True,
                        skip_group_check=True,
                    )
                # 1/denom = exp(-ln(denom)), one batched pass (Ln and Exp
                # share the natural_log_exp ACT table set).
                nc.scalar.activation(lnd_sb[:], den_sb[:], Ln)
                nc.scalar.activation(rec_sb[:], lnd_sb[:], Exp, scale=-1.0)

                # broadcast 1/denom across 64 partitions via K=1 matmuls,
                # then normalize ctx in place against the PSUM broadcast.
                for j in range(NH):
                    for c in range(QC):
                        cs = ts(c, 512)
                        recb_ps = psR.tile([64, 512], f32, tag="recb")
                        nc.tensor.matmul(
                            recb_ps,
                            ones_sb[32 * j : 32 * j + 1, 0:64],
                            rec_sb[32 * j : 32 * j + 1, cs],
                            start=True,
                            stop=True,
                        )
                        if j < 2:
                            ctx_slice = ctxA[j * 64 : (j + 1) * 64, cs]
                        else:
                            ctx_slice = ctxB[0:64, cs]
                        nc.vector.tensor_mul(ctx_slice, ctx_slice, recb_ps)

                for lt in range(KT):
                    y = yp.tile([128, D], f32, tag="y")
                    for n0, nw in ((0, 512), (512, 256)):
                        ps = ps3.tile([128, 512], f32, tag="ps3", name="ps3t")[:, :nw]
                        nc.tensor.matmul(
                            ps,
                            ctxA[:, ts(lt, 128)],
                            woutp_sb[:, 0, n0 : n0 + nw],
                            start=True,
                            stop=False,
                        )
                        nc.tensor.matmul(
                            ps,
                            ctxB[0:64, ts(lt, 128)],
                            woutp_sb[0:64, 1, n0 : n0 + nw],
                            start=False,
                            stop=True,
                        )
                        if n0 == 0:
                            nc.scalar.copy(y[:, n0 : n0 + nw], ps)
                        else:
                            nc.vector.tensor_copy(y[:, n0 : n0 + nw], ps)
                    nc.sync.dma_start(y_d[ts(lt, 128), :], y)

    if not nc.is_finalized():
        nc.finalize()
    _PROGRAM_CACHE["nc"] = nc
    return nc


def _host_inputs(x, attn_mask, Wqkv, bqkv, Wout, bout):
    """Build the 8 per-core input dicts."""
    import ml_dtypes

    bf16 = ml_dtypes.bfloat16
    slopes = alibi_slopes(H)
    x = np.asarray(x, dtype=np.float32)
    attn_mask = np.asarray(attn_mask)
    Wqkv = np.asarray(Wqkv, dtype=np.float32)
    bqkv = np.asarray(bqkv, dtype=np.float32)
    Wout = np.asarray(Wout, dtype=np.float32)
    bout = np.asarray(bout, dtype=np.float32)

    q_idx = np.arange(L, dtype=np.float32)
    ones_row = np.ones(L, dtype=np.float32)
    qrel = q_idx - QCENTER
    augqR = np.ascontiguousarray(np.stack([ones_row, ones_row, ones_row, -qrel]))
    augqL = np.ascontiguousarray(np.stack([ones_row, -ones_row, -ones_row, qrel]))

    # rel1[p, q''] = |q'' - p|  (kinked-segment relative distance)
    p = np.arange(128, dtype=np.float32)[:, None]
    qq = np.arange(128, dtype=np.float32)[None, :]
    rel1_base = np.abs(qq - p).astype(np.float32)

    in_maps = []
    for core in range(N_CORES):
        b = core // 4
        g = core % 4
        heads = HEAD_GROUPS[g]

        wqk = np.empty((D, 2 * DH * NH), np.float32)
        bqk = np.empty((128, NH), np.float32)
        wv = np.empty((D, DH * NH), np.float32)
        bv = np.empty((1, DH * NH), np.float32)
        woutp = np.zeros((256, D), np.float32)
        augk = np.empty((NH, 4, L), np.float32)
        mask_row = np.where(attn_mask[b] == 0, NEG_MASK, 0.0).astype(np.float32)
        rel1 = np.empty((128, NH, 128), np.float32)
        for jj, h in enumerate(heads):
            rel1[:, jj] = -float(slopes[h]) * rel1_base
            # Q half pre-scaled by 1/8 = 1/sqrt(DH)
            wqk[:, jj * 128 : jj * 128 + 64] = Wqkv[:, h * DH : (h + 1) * DH] * 0.125
            wqk[:, jj * 128 + 64 : (jj + 1) * 128] = Wqkv[
                :, D + h * DH : D + (h + 1) * DH
            ]
            bqk[0:64, jj] = bqkv[h * DH : (h + 1) * DH] * 0.125
            bqk[64:128, jj] = bqkv[D + h * DH : D + (h + 1) * DH]
            wv[:, jj * DH : (jj + 1) * DH] = Wqkv[
                :, 2 * D + h * DH : 2 * D + (h + 1) * DH
            ]
            bv[0, jj * DH : (jj + 1) * DH] = bqkv[2 * D + h * DH : 2 * D + (h + 1) * DH]
            woutp[jj * DH : (jj + 1) * DH, :] = Wout[h * DH : (h + 1) * DH, :]
            s = float(slopes[h])
            kterm = s * qrel  # s * (k_idx - 1024)
            hi = kterm.astype(bf16).astype(np.float32)
            augk[jj, 0, :] = mask_row
            augk[jj, 1, :] = hi
            augk[jj, 2, :] = kterm - hi
            augk[jj, 3, :] = s
        in_maps.append(
            {
                "xT": np.ascontiguousarray(x[b].T).astype(bf16),
                "wqk": wqk.astype(bf16),
                "bqk": bqk,
                "wv": wv.astype(bf16),
                "bv": bv.astype(bf16),
                "woutp": woutp.astype(bf16),
                "augqR": augqR,
                "augqL": augqL,
                "augk": augk,
                "rel1": rel1,
            }
        )
    return in_maps


def kernel(x, attn_mask, Wqkv, bqkv, Wout, bout):
    _ensure_concourse()
    from concourse.bass_utils import run_bass_kernel_spmd

    nc = _build_program()
    in_maps = _host_inputs(x, attn_mask, Wqkv, bqkv, Wout, bout)

    res = run_bass_kernel_spmd(
        nc,
        in_maps,
        list(range(N_CORES)),
        trace=bool(os.environ.get("BASS_TRACE")),
    )
    outs = [r["ypart"] for r in res.results]
    out = np.zeros((B, L, D), np.float32)
    for core in range(N_CORES):
        out[core // 4] += outs[core]
    out += np.asarray(bout, np.float32)[None, None, :]
    if res.exec_time_ns is not None:
        kernel.last_exec_time_ns = res.exec_time_ns
    kernel.last_result = res
    return out


# revision 51
# speedup vs baseline: 1.0269x; 1.0207x over previous
"""MultiHeadSelfAttention + ALiBi for Trainium2, SPMD over 8 NeuronCores.

Sharding: core c handles batch b = c // 4 and head group g = c % 4
(3 of the 12 heads, one per ALiBi band class so per-core work balances).
Each core computes y_partial[b] = ctx(heads_g) @ Wout[rows_g]; the host
sums the 4 partials per batch and adds bout.

Device pipeline per core:
  1. QKV projection in bf16 (weights/x pre-cast on host).  Q'/8+bq and
     K+bk written as float32r into per-head attention operand buffers;
     V (+ones column for softmax denominators) kept in bf16.
  2. S^T blocks [128k x 512q] as float32r matmuls with 4 augmented
     contraction rows carrying the attention-mask bias and, off the
     diagonal, the ALiBi term -slope*|q-k| (linear there; the k-index
     row is split hi+lo with hi bf16-exact so fp32r input rounding
     cannot distort it).  In diagonal blocks only the 128-wide kinked
     segment gets a fused DVE (rel * -slope + S) pass; the linear
     flanks use the aug-row matmul paths.  exp() on ScalarE over
     3-block groups -> bf16 P^T; P^T @ V_aug (bf16) accumulated in PSUM
     -> unnormalized ctx^T + denominator row.  Per-slot key-tile bands
     skip blocks where ALiBi decays attention below ~e^-14.
  3. Unnormalized ctx^T (bf16) + denominators staged to SBUF; one
     batched Ln + Exp(-x) pass produces 1/denom (single ACT table set);
     reciprocals broadcast across partitions via K=1 matmuls; ctx
     normalized in place; y = ctx^T.T @ Wout rows (bf16) -> fp32 out.
"""

import math
import os

import numpy as np


def _ensure_concourse():
    try:
        import concourse  # noqa: F401
    except ImportError:
        import sys

        for p in ("/opt/trn_rl_repo", "/root/.axon_site/_ro/trn_rl_repo"):
            if os.path.isdir(p) and p not in sys.path:
                sys.path.insert(0, p)


B, L, D, H, DH = 2, 2048, 768, 12, 64
KT = L // 128  # 16 k-tiles
QC = L // 512  # 4 q-chunks
NH = 3  # heads per core
N_CORES = 8
GROUP_SIZE = 3  # exp/S group size in k-tiles (3 PSUM banks)

NEG_MASK = -1.0e9
QCENTER = 1024.0  # index re-centering for fp32r aug rows

# Per head-slot key-tile bands per q-chunk (t_lo, t_hi_exclusive).  Heads
# are assigned to slots by band class; band d satisfies e^(-slope*d) <=
# e^-14 for every head in the slot, so skipped blocks contribute < 2e-3
# relative mass.  Slot 0: heads {7,6,5,4} (widest -> full); slot 1:
# heads {3,2,11,1} (d=224); slot 2: heads {10,9,0,8} (d=79).
BANDS = [
    [(0, 16), (0, 16), (0, 16), (0, 16)],  # slot 0: full
    [(0, 6), (2, 10), (6, 14), (10, 16)],  # slot 1: d=224
    [(0, 5), (3, 9), (7, 13), (11, 16)],  # slot 2: d=79
]

# One head per band class per group -> identical program on all cores.
HEAD_GROUPS = [[7, 3, 10], [6, 2, 9], [5, 11, 0], [4, 1, 8]]


def alibi_slopes(n_heads: int) -> np.ndarray:
    def slopes_pow2(n):
        start = 2 ** (-(2 ** -(math.log2(n) - 3)))
        return [start * start**i for i in range(n)]

    if math.log2(n_heads).is_integer():
        s = slopes_pow2(n_heads)
    else:
        cp = 2 ** int(math.floor(math.log2(n_heads)))
        s = slopes_pow2(cp) + slopes_pow2(2 * cp)[0::2][: n_heads - cp]
    return np.asarray(s, dtype=np.float32)


_PROGRAM_CACHE = {}


def _build_program():
    """Build the (shared, SPMD) Bass program once."""
    if "nc" in _PROGRAM_CACHE:
        return _PROGRAM_CACHE["nc"]

    _ensure_concourse()
    import concourse.mybir as mybir
    import concourse.tile as tile
    from concourse import bacc
    from concourse.bass import ts

    f32 = mybir.dt.float32
    f32r = mybir.dt.float32r
    bf16 = mybir.dt.bfloat16
    Exp = mybir.ActivationFunctionType.Exp
    Ln = mybir.ActivationFunctionType.Ln
    MULT = mybir.AluOpType.mult
    ADD = mybir.AluOpType.add

    nc = bacc.Bacc(None)

    # ---- DRAM I/O ----
    xT_d = nc.dram_tensor("xT", [D, L], bf16, kind="ExternalInput")
    wqk_d = nc.dram_tensor("wqk", [D, 2 * DH * NH], bf16, kind="ExternalInput")
    bqk_d = nc.dram_tensor("bqk", [128, NH], f32, kind="ExternalInput")
    wv_d = nc.dram_tensor("wv", [D, DH * NH], bf16, kind="ExternalInput")
    bv_d = nc.dram_tensor("bv", [1, DH * NH], bf16, kind="ExternalInput")
    woutp_d = nc.dram_tensor("woutp", [256, D], bf16, kind="ExternalInput")
    augqR_d = nc.dram_tensor("augqR", [4, L], f32r, kind="ExternalInput")
    augqL_d = nc.dram_tensor("augqL", [4, L], f32r, kind="ExternalInput")
    augk_d = nc.dram_tensor("augk", [NH, 4, L], f32r, kind="ExternalInput")
    # rel1[p, j, q''] = -slope_j * |q'' - p|: ALiBi for the 128-wide kinked
    # segment of diagonal blocks (the linear flanks use the aug rows).
    rel1_d = nc.dram_tensor("rel1", [128, NH, 128], f32, kind="ExternalInput")
    y_d = nc.dram_tensor("ypart", [L, D], f32, kind="ExternalOutput")

    with tile.TileContext(nc) as tc:
        with tc.tile_pool(name="persist", bufs=1) as pp:
            # ---- persistent SBUF ----
            wqk_sb = pp.tile([128, 6, 2 * DH * NH], bf16)
            bqk_sb = pp.tile([128, NH], f32)
            wv_sb = pp.tile([128, 6, DH * NH], bf16)
            bv_sb = pp.tile([1, DH * NH], bf16)
            woutp_sb = pp.tile([128, 2, D], bf16)
            rel1_sb = pp.tile([128, NH, 128], f32)
            ones_sb = pp.tile([65, 128], bf16)  # rows 0/32/64 used
            V_sb = pp.tile([128, KT, NH, DH + 1], bf16)
            ctxA = pp.tile([128, L], bf16)  # heads 0,1 of group (unnorm ctx^T)
            ctxB = pp.tile([64, L], bf16)  # head 2 of group
            # softmax denominators / reciprocals: head j lives on partition
            # 32*j so the K=1 broadcast matmul sees a legal base partition.
            den_sb = pp.tile([65, L], f32)
            lnd_sb = pp.tile([65, L], f32)
            rec_sb = pp.tile([65, L], bf16)
            # Per-head attention operand buffers (fp32r).
            # The ALiBi k-term is split hi+lo (hi bf16-exact) so the PE's
            # reduced-precision fp32r input rounding cannot distort it.
            # Kbuf rows: 0-63 K^T, 64 mask, 65 hi(s*(k-1024)), 66 lo, 67 s
            # QbufR rows: 0-63 Q'^T, 64 1, 65 1, 66 1, 67 -(q-1024)
            # QbufL rows: 0-63 Q'^T, 64 1, 65 -1, 66 -1, 67 +(q-1024)
            Kbuf = [
                pp.tile([68, L], f32r, tag=f"kb{j}", name=f"kb{j}") for j in range(NH)
            ]
            QbufR = [
                pp.tile([68, L], f32r, tag=f"qr{j}", name=f"qr{j}") for j in range(NH)
            ]
            QbufL = [
                pp.tile([68, L], f32r, tag=f"ql{j}", name=f"ql{j}") for j in range(NH)
            ]

            # Stage-1-critical DMAs first (x/wqk feed the first matmuls,
            # interleaved so the kt=0 pair lands earliest); later-needed
            # tensors go on the gpsimd DMA queue so they don't delay the
            # compute-critical loads.
            wqk_r = wqk_d.rearrange("(o p) m -> p o m", p=128)
            xT_r = xT_d.rearrange("(o p) f -> p o f", p=128)
            xT_sb = pp.tile([128, 6, L], bf16, name="xT_sb")
            for kt in range(6):
                nc.sync.dma_start(xT_sb[:, kt, :], xT_r[:, kt, :])
                nc.sync.dma_start(wqk_sb[:, kt, :], wqk_r[:, kt, :])
            nc.sync.dma_start(bqk_sb[:], bqk_d[:])
            nc.sync.dma_start(wv_sb[:], wv_d.rearrange("(o p) m -> p o m", p=128))
            nc.sync.dma_start(bv_sb[:], bv_d[:])
            nc.vector.memset(ones_sb[:], 1.0)
            nc.gpsimd.memset(V_sb[:, :, :, DH : DH + 1], 1.0)
            # unused den rows must stay finite through the batched Ln/Exp
            nc.gpsimd.memset(den_sb[:], 1.0)
            for j in range(NH):
                nc.gpsimd.dma_start(QbufR[j][64:68, :], augqR_d[:])
                nc.gpsimd.dma_start(QbufL[j][64:68, :], augqL_d[:])
                nc.gpsimd.dma_start(Kbuf[j][64:68, :], augk_d[j])
            nc.gpsimd.dma_start(rel1_sb[:], rel1_d[:])
            nc.gpsimd.dma_start(
                woutp_sb[:], woutp_d.rearrange("(o p) n -> p o n", p=128)
            )

            # Pre-touch DMA/memset-filled tiles with the engine that later
            # writes other rows of the same tile, so those writes carry a
            # single sync wait (walrus TS encoding rejects multi-wait).
            junk = pp.tile([1, 16], f32, name="junk")
            nc.vector.tensor_copy(junk[0:1, 0:1], bqk_sb[0:1, 0:1])
            nc.vector.tensor_copy(junk[0:1, 1:2], rel1_sb[0:1, 0, 0:1])
            junkr = pp.tile([1, 16], f32r, name="junkr")
            for j in range(NH):
                nc.vector.tensor_copy(junkr[0:1, 2 + j : 3 + j], QbufR[j][64:65, 0:1])
                nc.vector.tensor_copy(junkr[0:1, 5 + j : 6 + j], QbufL[j][64:65, 0:1])
                nc.vector.tensor_copy(junkr[0:1, 8 + j : 9 + j], Kbuf[j][64:65, 0:1])
            junkb = pp.tile([1, 4], bf16, name="junkb")
            nc.vector.tensor_copy(junkb[0:1, 0:1], V_sb[0:1, 0, 0, DH : DH + 1])

            # ---- stage 1: QKV projection (bf16 matmuls) ----
            with tc.tile_pool(name="ps1", bufs=4, space="PSUM") as ps1:
                # Q^T/K^T per head: PSUM [128, 512] = [Q^T_h/8; K^T_h] chunk
                # (the 1/8 scale is folded into wqk/bqk host-side)
                for j in range(NH):
                    pcs = [
                        ps1.tile([128, 512], f32, tag="ps1", name=f"ps1c{c}")
                        for c in range(QC)
                    ]
                    for kt in range(6):
                        for c in range(QC):
                            nc.tensor.matmul(
                                pcs[c],
                                wqk_sb[:, kt, ts(j, 128)],
                                xT_sb[:, kt, ts(c, 512)],
                                start=(kt == 0),
                                stop=(kt == 5),
                            )
                    for c in range(QC):
                        ps = pcs[c]
                        cs = ts(c, 512)
                        nc.vector.tensor_scalar(
                            QbufR[j][0:64, cs],
                            ps[0:64, :],
                            bqk_sb[0:64, j : j + 1],
                            None,
                            ADD,
                        )
                        # Q rows are identical in both side-buffers; replicate
                        # via DMA to keep the DVE free.
                        nc.sync.dma_start(QbufL[j][0:64, cs], QbufR[j][0:64, cs])
                        nc.vector.tensor_scalar(
                            Kbuf[j][0:64, cs],
                            ps[64:128, :],
                            bqk_sb[64:128, j : j + 1],
                            None,
                            ADD,
                        )

                # V natural layout [l, d] + bias via K=1 matmul
                for lt in range(KT):
                    psv = ps1.tile([128, 512], f32, tag="ps1", name="psv")[:, : DH * NH]
                    for kt in range(6):
                        nc.tensor.matmul(
                            psv,
                            xT_sb[:, kt, ts(lt, 128)],
                            wv_sb[:, kt, :],
                            start=(kt == 0),
                            stop=False,
                        )
                    nc.tensor.matmul(
                        psv,
                        ones_sb[0:1, 0:128],
                        bv_sb[0:1, :],
                        start=False,
                        stop=True,
                    )
                    nc.vector.tensor_copy(
                        V_sb[:, lt, :, 0:DH],
                        psv.rearrange("p (h x) -> p h x", x=DH),
                    )

            # ---- stage 2: attention ----
            with (
                tc.tile_pool(name="psS", bufs=2, space="PSUM") as psS,
                tc.tile_pool(name="psO", bufs=2, space="PSUM") as psO,
                tc.tile_pool(name="ptp", bufs=4) as ptp,
            ):
                for j in range(NH):
                    for c in range(QC):
                        cs = ts(c, 512)
                        t_lo, t_hi = BANDS[j][c]
                        out_t = psO.tile([128, 512], f32, tag="outaug")
                        for t0 in range(t_lo, t_hi, GROUP_SIZE):
                            tn = min(GROUP_SIZE, t_hi - t0)
                            st = psS.tile([128, GROUP_SIZE * 512], f32, tag="st")
                            for i in range(tn):
                                t = t0 + i
                                js = ts(i, 512)
                                m = t - 4 * c
                                if 0 <= m < 4:
                                    # Diagonal block: the abs kink only hits
                                    # q' in [128m, 128m+128); the flanks are
                                    # linear and use the aug-row paths.
                                    lw = 128 * m  # flank where q < k
                                    for a, b, buf, nr in (
                                        (0, lw, QbufL[j], 68),
                                        (lw, lw + 128, QbufR[j], 65),
                                        (lw + 128, 512, QbufR[j], 68),
                                    ):
                                        if a == b:
                                            continue
                                        nc.tensor.matmul(
                                            st[:, 512 * i + a : 512 * i + b],
                                            Kbuf[j][0:nr, ts(t, 128)],
                                            buf[0:nr, 512 * c + a : 512 * c + b],
                                            start=True,
                                            stop=True,
                                        )
                                    ms = slice(512 * i + lw, 512 * i + lw + 128)
                                    nc.vector.scalar_tensor_tensor(
                                        st[:, ms],
                                        rel1_sb[:, j, :],
                                        1.0,
                                        st[:, ms],
                                        MULT,
                                        ADD,
                                    )
                                elif c > t // 4:  # k < q: aug gives -s*(q-k)
                                    nc.tensor.matmul(
                                        st[:, js],
                                        Kbuf[j][0:68, ts(t, 128)],
                                        QbufR[j][0:68, cs],
                                        start=True,
                                        stop=True,
                                    )
                                else:  # k > q: aug gives -s*(k-q)
                                    nc.tensor.matmul(
                                        st[:, js],
                                        Kbuf[j][0:68, ts(t, 128)],
                                        QbufL[j][0:68, cs],
                                        start=True,
                                        stop=True,
                                    )
                            pt = ptp.tile([128, GROUP_SIZE * 512], bf16, tag="pt")
                            nc.scalar.activation(
                                pt[:, : tn * 512], st[:, : tn * 512], Exp
                            )
                            for i in range(tn):
                                t = t0 + i
                                nc.tensor.matmul(
                                    out_t[0 : DH + 1, :],
                                    V_sb[:, t, j, :],
                                    pt[:, ts(i, 512)],
                                    start=(t == t_lo),
                                    stop=(t == t_hi - 1),
                                    skip_group_check=True,
                                )
                        # stage unnormalized ctx + denominator to SBUF
                        if j < 2:
                            ctx_slice = ctxA[j * 64 : (j + 1) * 64, cs]
                        else:
                            ctx_slice = ctxB[0:64, cs]
                        nc.vector.tensor_copy(ctx_slice, out_t[0:DH, :])
                        nc.vector.tensor_copy(
                            den_sb[32 * j : 32 * j + 1, cs], out_t[DH : DH + 1, :]
                        )

            # ---- stage 3: normalization + output projection ----
            with (
                tc.tile_pool(name="ps3", bufs=2, space="PSUM") as ps3,
                tc.tile_pool(name="psR", bufs=2, space="PSUM") as psR,
                tc.tile_pool(name="ysb", bufs=3) as yp,
            ):
                # Keep the PE busy (HAM warm) through the reciprocal chain:
                # these matmuls depend on den_sb, so they schedule exactly
                # into the Ln/Exp window the PE would otherwise idle in.
                # (read the last-written den row/chunk so these only become
                # ready once stage 2 fully drains; bf16 bitcast keeps each
                # at ~213ns so the total matches the Ln/Exp latency)
                dum = psR.tile([128, 512], f32, tag="dum")
                dk = den_sb[64:65, 1536:2048].bitcast(bf16)
                for _ in range(16):
                    nc.tensor.matmul(
                        dum,
                        dk[0:1, 0:128],
                        dk[0:1, 0:512],
                        start=True,
                        stop=True,
                        skip_group_check=True,
                    )
                # 1/denom = exp(-ln(denom)), one batched pass (Ln and Exp
                # share the natural_log_exp ACT table set).
                nc.scalar.activation(lnd_sb[:], den_sb[:], Ln)
                nc.scalar.activation(rec_sb[:], lnd_sb[:], Exp, scale=-1.0)

                # broadcast 1/denom across 64 partitions via K=1 matmuls,
                # then normalize ctx in place against the PSUM broadcast.
                for j in range(NH):
                    for c in range(QC):
                        cs = ts(c, 512)
                        recb_ps = psR.tile([64, 512], f32, tag="recb")
                        nc.tensor.matmul(
                            recb_ps,
                            ones_sb[32 * j : 32 * j + 1, 0:64],
                            rec_sb[32 * j : 32 * j + 1, cs],
                            start=True,
                            stop=True,
                        )
                        if j < 2:
                            ctx_slice = ctxA[j * 64 : (j + 1) * 64, cs]
                        else:
                            ctx_slice = ctxB[0:64, cs]
                        nc.vector.tensor_mul(ctx_slice, ctx_slice, recb_ps)

                for lt in range(KT):
                    y = yp.tile([128, D], f32, tag="y")
                    for n0, nw in ((0, 512), (512, 256)):
                        ps = ps3.tile([128, 512], f32, tag="ps3", name="ps3t")[:, :nw]
                        nc.tensor.matmul(
                            ps,
                            ctxA[:, ts(lt, 128)],
                            woutp_sb[:, 0, n0 : n0 + nw],
                            start=True,
                            stop=False,
                        )
                        nc.tensor.matmul(
                            ps,
                            ctxB[0:64, ts(lt, 128)],
                            woutp_sb[0:64, 1, n0 : n0 + nw],
                            start=False,
                            stop=True,
                        )
                        if n0 == 0:
                            nc.scalar.copy(y[:, n0 : n0 + nw], ps)
                        else:
                            nc.vector.tensor_copy(y[:, n0 : n0 + nw], ps)
                    nc.sync.dma_start(y_d[ts(lt, 128), :], y)

    if not nc.is_finalized():
        nc.finalize()
    _PROGRAM_CACHE["nc"] = nc
    return nc


def _host_inputs(x, attn_mask, Wqkv, bqkv, Wout, bout):
    """Build the 8 per-core input dicts."""
    import ml_dtypes

    bf16 = ml_dtypes.bfloat16
    slopes = alibi_slopes(H)
    x = np.asarray(x, dtype=np.float32)
    attn_mask = np.asarray(attn_mask)
    Wqkv = np.asarray(Wqkv, dtype=np.float32)
    bqkv = np.asarray(bqkv, dtype=np.float32)
    Wout = np.asarray(Wout, dtype=np.float32)
    bout = np.asarray(bout, dtype=np.float32)

    q_idx = np.arange(L, dtype=np.float32)
    ones_row = np.ones(L, dtype=np.float32)
    qrel = q_idx - QCENTER
    augqR = np.ascontiguousarray(np.stack([ones_row, ones_row, ones_row, -qrel]))
    augqL = np.ascontiguousarray(np.stack([ones_row, -ones_row, -ones_row, qrel]))

    # rel1[p, q''] = |q'' - p|  (kinked-segment relative distance)
    p = np.arange(128, dtype=np.float32)[:, None]
    qq = np.arange(128, dtype=np.float32)[None, :]
    rel1_base = np.abs(qq - p).astype(np.float32)

    in_maps = []
    for core in range(N_CORES):
        b = core // 4
        g = core % 4
        heads = HEAD_GROUPS[g]

        wqk = np.empty((D, 2 * DH * NH), np.float32)
        bqk = np.empty((128, NH), np.float32)
        wv = np.empty((D, DH * NH), np.float32)
        bv = np.empty((1, DH * NH), np.float32)
        woutp = np.zeros((256, D), np.float32)
        augk = np.empty((NH, 4, L), np.float32)
        mask_row = np.where(attn_mask[b] == 0, NEG_MASK, 0.0).astype(np.float32)
        rel1 = np.empty((128, NH, 128), np.float32)
        for jj, h in enumerate(heads):
            rel1[:, jj] = -float(slopes[h]) * rel1_base
            # Q half pre-scaled by 1/8 = 1/sqrt(DH)
            wqk[:, jj * 128 : jj * 128 + 64] = Wqkv[:, h * DH : (h + 1) * DH] * 0.125
            wqk[:, jj * 128 + 64 : (jj + 1) * 128] = Wqkv[
                :, D + h * DH : D + (h + 1) * DH
            ]
            bqk[0:64, jj] = bqkv[h * DH : (h + 1) * DH] * 0.125
            bqk[64:128, jj] = bqkv[D + h * DH : D + (h + 1) * DH]
            wv[:, jj * DH : (jj + 1) * DH] = Wqkv[
                :, 2 * D + h * DH : 2 * D + (h + 1) * DH
            ]
            bv[0, jj * DH : (jj + 1) * DH] = bqkv[2 * D + h * DH : 2 * D + (h + 1) * DH]
            woutp[jj * DH : (jj + 1) * DH, :] = Wout[h * DH : (h + 1) * DH, :]
            s = float(slopes[h])
            kterm = s * qrel  # s * (k_idx - 1024)
            hi = kterm.astype(bf16).astype(np.float32)
            augk[jj, 0, :] = mask_row
            augk[jj, 1, :] = hi
            augk[jj, 2, :] = kterm - hi
            augk[jj, 3, :] = s
        in_maps.append(
            {
                "xT": np.ascontiguousarray(x[b].T).astype(bf16),
                "wqk": wqk.astype(bf16),
                "bqk": bqk,
                "wv": wv.astype(bf16),
                "bv": bv.astype(bf16),
                "woutp": woutp.astype(bf16),
                "augqR": augqR,
                "augqL": augqL,
                "augk": augk,
                "rel1": rel1,
            }
        )
    return in_maps


def kernel(x, attn_mask, Wqkv, bqkv, Wout, bout):
    _ensure_concourse()
    from concourse.bass_utils import run_bass_kernel_spmd

    nc = _build_program()
    in_maps = _host_inputs(x, attn_mask, Wqkv, bqkv, Wout, bout)

    res = run_bass_kernel_spmd(
        nc,
        in_maps,
        list(range(N_CORES)),
        trace=bool(os.environ.get("BASS_TRACE")),
    )
    outs = [r["ypart"] for r in res.results]
    out = np.zeros((B, L, D), np.float32)
    for core in range(N_CORES):
        out[core // 4] += outs[core]
    out += np.asarray(bout, np.float32)[None, None, :]
    if res.exec_time_ns is not None:
        kernel.last_exec_time_ns = res.exec_time_ns
    kernel.last_result = res
    return out
